# revision 18
# baseline (speedup 1.0000x reference)
"""DeepseekV3 sparse attention on 8 Trainium2 NeuronCores — full on-device.

Sharding: KV path row-sharded (256 contiguous rows/core) + AllGather of
K^T/V/k_pe^T/ki^T; query side stride-8 interleaved (2 uniform 128-row tiles
per core, causal extents 1024/2048 — identical SPMD structure on all cores);
indexer + top-k (20-iter per-row bisection) + sparse attention + Wo fully
on-device. Output rows gathered on host.
"""

import os
import sys

sys.path.insert(0, "/opt/trn_rl_repo")

import numpy as np
import ml_dtypes

BF = ml_dtypes.bfloat16
B, S, H = 1, 2048, 2048
QL, KVL = 1536, 512
NH, NOPE, ROPE, VD = 16, 128, 64, 128
IH, ID = 16, 128
TOPK = 512
EPS = 1e-6
NC = 8
SCALE = float((NOPE + ROPE) ** -0.5)
BISECT_ITERS = 20
EXTS = (1024, 2048)

_cached = {}


def _build(debug=False):
    import concourse.mybir as mybir
    from concourse import bacc
    from concourse.tile import TileContext

    F32 = mybir.dt.float32
    BF16 = mybir.dt.bfloat16
    AF = mybir.ActivationFunctionType
    OP = mybir.AluOpType

    nc = bacc.Bacc(num_devices=NC)

    def inp(name, shape, dt=F32):
        return nc.dram_tensor(name, shape, dt, kind="ExternalInput")

    # per-core inputs
    hsq = inp("hsq", [H, 256])            # hs^T cols at q rows (tile0|tile1)
    hskv = inp("hskv", [H, 256])          # hs^T cols at kv rows (contiguous)
    hskvb = inp("hskvb", [H, 256], BF16)
    cosq = inp("cosq", [64, 256])
    sinq = inp("sinq", [64, 256])
    coskv = inp("coskv", [64, 256])
    sinkv = inp("sinkv", [64, 256])
    causal = inp("causal", [128, 1024])
    # shared weights
    wqa = inp("wqa", [H, QL])
    qga = inp("qga", [128, 12])
    wqb = inp("wqb", [QL, 3072], BF16)     # reordered: 16 nope chunks + 8 rope pairs
    wkva = inp("wkva", [H, 576], BF16)
    kvga = inp("kvga", [128, 4])
    wkvb = inp("wkvb", [KVL, 4096], BF16)
    wo = inp("wo", [NH * VD, H], BF16)
    wqi = inp("wqi", [QL, IH * ID])
    wki = inp("wki", [H, ID])
    wwi = inp("wwi", [H, IH])              # pre-scaled by ID**-0.5
    knga = inp("knga", [128, 1])
    knbe = inp("knbe", [128, 1])
    ones_c = inp("ones_c", [128, 1])       # fp32 ones column (lhsT partition sums)
    ones_r = inp("ones_r", [1, 128])       # fp32 ones row (lhsT broadcasts)
    ident = inp("ident", [128, 128], BF16)
    cnsts = inp("cnsts", [128, 2])         # cols: [0.0, EPS]

    y = nc.dram_tensor("y", [2, 128, H], F32, kind="ExternalOutput")
    dbg = {}
    if debug:
        dbg["qaT0"] = nc.dram_tensor("dbg_qaT0", [128, 256], F32, kind="ExternalOutput")
        dbg["idx1"] = nc.dram_tensor("dbg_idx1", [128, 2048], F32, kind="ExternalOutput")
        dbg["lo1"] = nc.dram_tensor("dbg_lo1", [128, 1], F32, kind="ExternalOutput")
        dbg["sel1"] = nc.dram_tensor("dbg_sel1", [128, 2048], BF16, kind="ExternalOutput")
        dbg["att10"] = nc.dram_tensor("dbg_att10", [128, 128], F32, kind="ExternalOutput")
        dbg["ki"] = nc.dram_tensor("dbg_ki", [NC * ID, 256], F32, kind="ExternalOutput")
        dbg["kpe"] = nc.dram_tensor("dbg_kpe", [NC * 64, 256], F32, kind="ExternalOutput")
        dbg["kt"] = nc.dram_tensor("dbg_kt", [NC * NH * NOPE, 256], BF16, kind="ExternalOutput")
        dbg["vt"] = nc.dram_tensor("dbg_vt", [NC * 256, NH * VD], BF16, kind="ExternalOutput")
        dbg["wt"] = nc.dram_tensor("dbg_wt", [128, 16], F32, kind="ExternalOutput")
        dbg["qi3"] = nc.dram_tensor("dbg_qi3", [128, 256], F32, kind="ExternalOutput")

    # internal DRAM (collective bounce)
    kT_sh = nc.dram_tensor("kT_sh", [NH * NOPE, 256], BF16)
    kT_full = nc.dram_tensor("kT_full", [NC * NH * NOPE, 256], BF16, addr_space="Shared")
    v_sh = nc.dram_tensor("v_sh", [256, NH * VD], BF16)
    v_full = nc.dram_tensor("v_full", [NC * 256, NH * VD], BF16, addr_space="Shared")
    kpe_sh = nc.dram_tensor("kpe_sh", [64, 256], F32)
    kpe_full = nc.dram_tensor("kpe_full", [NC * 64, 256], F32, addr_space="Shared")
    ki_sh = nc.dram_tensor("ki_sh", [ID, 256], F32)
    ki_full = nc.dram_tensor("ki_full", [NC * ID, 256], F32, addr_space="Shared")

    RG = [list(range(NC))]

    with TileContext(nc) as tc:
        with (
            tc.tile_pool(name="pc", bufs=1) as pc,       # consts + residents
            tc.tile_pool(name="pw", bufs=3) as pw,       # streamed weights
            tc.tile_pool(name="px", bufs=2) as px,       # transient work
        ):
            dma = nc.sync.dma_start
            V = nc.vector
            A = nc.scalar
            MM = nc.tensor.matmul

            # ---- resident loads ----
            def ld(name, shape, dt, src_ap):
                t = pc.tile(shape, dt, tag=name, name=name)
                dma(out=t[:], in_=src_ap)
                return t

            cosq_sb = ld("cosq", [64, 256], F32, cosq.ap())
            sinq_sb = ld("sinq", [64, 256], F32, sinq.ap())
            coskv_sb = ld("coskv", [64, 256], F32, coskv.ap())
            sinkv_sb = ld("sinkv", [64, 256], F32, sinkv.ap())
            causal_sb = ld("causal", [128, 1024], F32, causal.ap())
            qga_sb = ld("qga", [128, 12], F32, qga.ap())
            kvga_sb = ld("kvga", [128, 4], F32, kvga.ap())
            knga_sb = ld("knga", [128, 1], F32, knga.ap())
            knbe_sb = ld("knbe", [128, 1], F32, knbe.ap())
            ones_c_sb = ld("ones_c", [128, 1], F32, ones_c.ap())
            ones_r_sb = ld("ones_r", [1, 128], F32, ones_r.ap())
            ident_sb = ld("ident", [128, 128], BF16, ident.ap())
            wki_sb = ld("wki", [128, 16, ID], F32,
                        wki.ap().rearrange("(c p) n -> p c n", p=128))
            wwi_sb = ld("wwi", [128, 16, IH], F32,
                        wwi.ap().rearrange("(c p) n -> p c n", p=128))
            cnsts_sb = ld("cnsts", [128, 2], F32, cnsts.ap())
            nc.const_aps.aps[(F32, 0.0)] = cnsts_sb[:, 0:1]
            nc.const_aps.aps[(F32, EPS)] = cnsts_sb[:, 1:2]

            def rope(dst64, src64, cos_t, sin_pm):
                """dst = src*cosT2 + swap32(src)*sin_pm; all base-0 [64,256]."""
                xs = px.tile([64, 256], F32, tag="xs", name="xs", bufs=2)
                dma(out=xs[0:32], in_=src64[32:64])
                dma(out=xs[32:64], in_=src64[0:32])
                t1 = px.tile([64, 256], F32, tag="rope_t1", name="t1")
                t2 = px.tile([64, 256], F32, tag="rope_t2", name="t2")
                V.tensor_mul(t1[:], src64, cos_t)
                V.tensor_mul(t2[:], xs[:], sin_pm)
                V.tensor_add(dst64, t1[:], t2[:])

            # ================= PHASE K1: kv_a + norms + ki + kpe =================
            kvcb = [pc.tile([128, 256], BF16, tag=f"kvcb{m}", name=f"kvcb{m}")
                    for m in range(4)]
            with tc.tile_pool(name="ppK1", bufs=1, space="PSUM") as pk:
                ps_kv = [pk.tile([128, 256], F32, tag=f"pkv{m}", name=f"pkv{m}")
                         for m in range(4)]
                ps_pe = pk.tile([64, 256], F32, tag="pkpe")
                ps_ki = pk.tile([128, 256], F32, tag="pki")
                ss_ps = pk.tile([1, 256], F32, tag="pss")
                for hc in range(16):
                    wkva_t = pw.tile([128, 576], BF16, tag="wkva", name="wkva_t")
                    dma(out=wkva_t[:], in_=wkva[hc * 128:(hc + 1) * 128, :])
                    hkb_t = pw.tile([128, 256], BF16, tag="hstb", bufs=4,
                                    name="hkb_t")
                    dma(out=hkb_t[:], in_=hskvb[hc * 128:(hc + 1) * 128, :])
                    hkf_t = pw.tile([128, 256], F32, tag="hst", bufs=4,
                                    name="hkf_t")
                    dma(out=hkf_t[:], in_=hskv[hc * 128:(hc + 1) * 128, :])
                    for m in range(4):
                        MM(ps_kv[m][:], wkva_t[:, m * 128:(m + 1) * 128],
                           hkb_t[:], start=(hc == 0), stop=(hc == 15))
                    MM(ps_pe[:], wkva_t[:, 512:576], hkb_t[:],
                       start=(hc == 0), stop=(hc == 15))
                    MM(ps_ki[:], wki_sb[:, hc, :], hkf_t[:],
                       start=(hc == 0), stop=(hc == 15))
                # rms over kv_c (partition-dim sums via ones-matmul)
                kvc_raw = [px.tile([128, 256], F32, tag=f"kvcr{m}", bufs=1,
                                   name=f"kvcr{m}") for m in range(4)]
                for m in range(4):
                    V.tensor_copy(kvc_raw[m][:], ps_kv[m][:])
                    sq = px.tile([128, 256], F32, tag="sq", name="sq")
                    A.square(sq[:], kvc_raw[m][:])
                    MM(ss_ps[:], ones_c_sb[:], sq[:], start=(m == 0), stop=(m == 3))
                sd = px.tile([1, 256], F32, tag="sd", name="sd")
                A.activation(sd[:], ss_ps[:], AF.Sqrt, bias=EPS, scale=1.0 / KVL)
                iv = px.tile([1, 256], F32, tag="iv", name="iv")
                V.reciprocal(iv[:], sd[:])
                ivb_ps = pk.tile([128, 256], F32, tag="ptmp")
                MM(ivb_ps[:], ones_r_sb[:], iv[:], start=True, stop=True)
                ivb = px.tile([128, 256], F32, tag="ivb", name="ivb")
                V.tensor_copy(ivb[:], ivb_ps[:])
                for m in range(4):
                    V.scalar_tensor_tensor(kvcb[m][:], kvc_raw[m][:],
                                           kvga_sb[:, m:m + 1], ivb[:],
                                           op0=OP.mult, op1=OP.mult)
                # k_pe rope -> shard
                kpe_t = px.tile([64, 256], F32, tag="kpe_t", name="kpe_t")
                V.tensor_copy(kpe_t[:], ps_pe[:])
                kpe_o = px.tile([64, 256], F32, tag="kpe_o", name="kpe_o")
                rope(kpe_o[:], kpe_t[:], coskv_sb[:], sinkv_sb[:])
                dma(out=kpe_sh[:, :], in_=kpe_o[:])
                # ki layer norm + rope -> shard
                ki_raw = px.tile([128, 256], F32, tag="ki_raw", bufs=1, name="ki_raw")
                V.tensor_copy(ki_raw[:], ps_ki[:])
                mu_ps = pk.tile([1, 256], F32, tag="ptmp")
                MM(mu_ps[:], ones_c_sb[:], ki_raw[:], start=True, stop=True)
                mu = px.tile([1, 256], F32, tag="mu", name="mu")
                A.activation(mu[:], mu_ps[:], AF.Copy, scale=1.0 / ID)
                mub_ps = pk.tile([128, 256], F32, tag="ptmp")
                MM(mub_ps[:], ones_r_sb[:], mu[:], start=True, stop=True)
                xc = px.tile([128, 256], F32, tag="xc", bufs=1, name="xc")
                V.tensor_sub(xc[:], ki_raw[:], mub_ps[:])
                sq2 = px.tile([128, 256], F32, tag="sq", name="sq2")
                A.square(sq2[:], xc[:])
                vv_ps = pk.tile([1, 256], F32, tag="ptmp")
                MM(vv_ps[:], ones_c_sb[:], sq2[:], start=True, stop=True)
                sd2 = px.tile([1, 256], F32, tag="sd", name="sd2")
                A.activation(sd2[:], vv_ps[:], AF.Sqrt, bias=EPS, scale=1.0 / ID)
                iv2 = px.tile([1, 256], F32, tag="iv", name="iv2")
                V.reciprocal(iv2[:], sd2[:])
                ivb2_ps = pk.tile([128, 256], F32, tag="ptmp")
                MM(ivb2_ps[:], ones_r_sb[:], iv2[:], start=True, stop=True)
                ivb2 = px.tile([128, 256], F32, tag="ivb", name="ivb2")
                V.tensor_copy(ivb2[:], ivb2_ps[:])
                kin = px.tile([128, 256], F32, tag="kin", bufs=1, name="kin")
                V.scalar_tensor_tensor(kin[:], xc[:], knga_sb[:], ivb2[:],
                                       op0=OP.mult, op1=OP.mult)
                V.tensor_scalar(kin[:], kin[:], knbe_sb[:], None, op0=OP.add)
                ki_out = px.tile([128, 256], F32, tag="ki_out", bufs=1, name="ki_out")
                rope(ki_out[0:64], kin[0:64], coskv_sb[:], sinkv_sb[:])
                V.tensor_copy(ki_out[64:128], kin[64:128])
                dma(out=ki_sh[:, :], in_=ki_out[:])

            # ================= PHASE K2: k^T and v shards =================
            with tc.tile_pool(name="ppK2", bufs=1, space="PSUM") as pk2:
                for h in range(NH):
                    wkvb_t = [pw.tile([128, 256], BF16, tag=f"wkvbt{k}",
                                      name=f"wkvbt{k}", bufs=2) for k in range(4)]
                    for k in range(4):
                        dma(out=wkvb_t[k][:],
                            in_=wkvb[k * 128:(k + 1) * 128, h * 256:(h + 1) * 256])
                    ps_k = pk2.tile([128, 256], F32, tag="pkT", bufs=2)
                    for k in range(4):
                        MM(ps_k[:], wkvb_t[k][:, 0:128], kvcb[k][:],
                           start=(k == 0), stop=(k == 3))
                    kt = px.tile([128, 256], BF16, tag="kt", name="kt", bufs=2)
                    A.copy(kt[:], ps_k[:])
                    dma(out=kT_sh[h * 128:(h + 1) * 128, :], in_=kt[:])
                    for ss in range(2):
                        ps_v = pk2.tile([128, 128], F32, tag=f"pv{ss}", bufs=2)
                        for k in range(4):
                            MM(ps_v[:], kvcb[k][:, ss * 128:(ss + 1) * 128],
                               wkvb_t[k][:, 128:256], start=(k == 0), stop=(k == 3))
                        vt = px.tile([128, 128], BF16, tag="vt", name="vt", bufs=2)
                        A.copy(vt[:], ps_v[:])
                        dma(out=v_sh[ss * 128:(ss + 1) * 128, h * 128:(h + 1) * 128],
                            in_=vt[:])
            # collectives
            cc = nc.gpsimd.collective_compute
            cc("AllGather", mybir.AluOpType.bypass, RG,
               [kT_sh.ap().opt()], [kT_full.ap().opt()])
            cc("AllGather", mybir.AluOpType.bypass, RG,
               [v_sh.ap().opt()], [v_full.ap().opt()])
            cc("AllGather", mybir.AluOpType.bypass, RG,
               [kpe_sh.ap().opt()], [kpe_full.ap().opt()])
            cc("AllGather", mybir.AluOpType.bypass, RG,
               [ki_sh.ap().opt()], [ki_full.ap().opt()])

            # ================= PHASE Q1: q_a (fp32 + rms) =================
            qaraw = [pc.tile([128, 256], F32, tag=f"qaraw{q}", name=f"qaraw{q}")
                     for q in range(12)]
            qaTb = [pc.tile([128, 256], BF16, tag=f"qaTb{q}", name=f"qaTb{q}")
                    for q in range(12)]
            with tc.tile_pool(name="ppQ1", bufs=1, space="PSUM") as pq:
                ssq_ps = pq.tile([1, 256], F32, tag="pssq")
                for g in range(2):
                    ps_qa = [pq.tile([128, 256], F32, tag=f"pqa{q}",
                                     name=f"ps_qa{q}") for q in range(6)]
                    for hc in range(16):
                        wqa_t = pw.tile([128, 768], F32, tag="wqa", name="wqa_t", bufs=2)
                        dma(out=wqa_t[:], in_=wqa[hc * 128:(hc + 1) * 128,
                                                  g * 768:(g + 1) * 768])
                        hq_t = pw.tile([128, 256], F32, tag="hst", bufs=4,
                                       name="hq_t")
                        dma(out=hq_t[:], in_=hsq[hc * 128:(hc + 1) * 128, :])
                        for q in range(6):
                            MM(ps_qa[q][:], wqa_t[:, q * 128:(q + 1) * 128],
                               hq_t[:], start=(hc == 0), stop=(hc == 15))
                    for q in range(6):
                        qlc = g * 6 + q
                        V.tensor_copy(qaraw[qlc][:], ps_qa[q][:])
                        sq = px.tile([128, 256], F32, tag="sq", name="sqq")
                        A.square(sq[:], qaraw[qlc][:])
                        MM(ssq_ps[:], ones_c_sb[:], sq[:],
                           start=(qlc == 0), stop=(qlc == 11))
                sdq = px.tile([1, 256], F32, tag="sd", name="sdq")
                A.activation(sdq[:], ssq_ps[:], AF.Sqrt, bias=EPS, scale=1.0 / QL)
                ivq = px.tile([1, 256], F32, tag="iv", name="ivq")
                V.reciprocal(ivq[:], sdq[:])
                ivbq_ps = pq.tile([128, 256], F32, tag="ptmpq")
                MM(ivbq_ps[:], ones_r_sb[:], ivq[:], start=True, stop=True)
                ivbq = px.tile([128, 256], F32, tag="ivbq", bufs=1, name="ivbq")
                V.tensor_copy(ivbq[:], ivbq_ps[:])
                for q in range(12):
                    # in-place: qaraw becomes q_a^T (normed, gamma applied)
                    V.scalar_tensor_tensor(qaraw[q][:], qaraw[q][:],
                                           qga_sb[:, q:q + 1], ivbq[:],
                                           op0=OP.mult, op1=OP.mult)
                    A.copy(qaTb[q][:], qaraw[q][:])
            qaT = qaraw
            if debug:
                dma(out=dbg["qaT0"][:, :], in_=qaT[0][:])

            # ================= PHASE Q2: w, qi^T (w-folded, roped), q^T ========
            qiTw = [pc.tile([128, 256], F32, tag=f"qiT{h}", name=f"qiT{h}")
                    for h in range(IH)]
            qnT = [pc.tile([128, 256], BF16, tag=f"qnT{h}", name=f"qnT{h}")
                   for h in range(NH)]
            qpeT = [pc.tile([64, 256], BF16, tag=f"qpeT{h}", name=f"qpeT{h}")
                    for h in range(NH)]
            w_rm = [pc.tile([128, 16], F32, tag=f"w_rm{t}", name=f"w_rm{t}")
                    for t in range(2)]
            with tc.tile_pool(name="ppQ2", bufs=1, space="PSUM") as pq2:
                psw = [pq2.tile([128, 16], F32, tag=f"pw16{t}", name=f"psw{t}") for t in range(2)]
                for hc in range(16):
                    hq2_t = pw.tile([128, 256], F32, tag="hst", bufs=4,
                                    name="hq2_t")
                    dma(out=hq2_t[:], in_=hsq[hc * 128:(hc + 1) * 128, :])
                    for t in range(2):
                        MM(psw[t][:], hq2_t[:, t * 128:(t + 1) * 128],
                           wwi_sb[:, hc, :], start=(hc == 0), stop=(hc == 15))
                for t in range(2):
                    V.tensor_copy(w_rm[t][:], psw[t][:])
                if debug:
                    dma(out=dbg["wt"][:, :], in_=w_rm[0][:])
                for g in range(4):  # qi: 4 heads per group
                    ps_qi = [pq2.tile([128, 256], F32, tag=f"pqi{j}",
                                      name=f"ps_qi{j}") for j in range(4)]
                    for qlc in range(12):
                        wqi_t = pw.tile([128, 512], F32, tag="wqi", name="wqi_t")
                        dma(out=wqi_t[:], in_=wqi[qlc * 128:(qlc + 1) * 128,
                                                  g * 512:(g + 1) * 512])
                        for j in range(4):
                            MM(ps_qi[j][:], wqi_t[:, j * 128:(j + 1) * 128],
                               qaT[qlc][:], start=(qlc == 0), stop=(qlc == 11))
                    for j in range(4):
                        h = g * 4 + j
                        qi_s = px.tile([128, 256], F32, tag="qi_s", name="qi_s",
                                       bufs=2)
                        V.tensor_copy(qi_s[:], ps_qi[j][:])
                        rope(qiTw[h][0:64], qi_s[0:64], cosq_sb[:], sinq_sb[:])
                        V.tensor_copy(qiTw[h][64:128], qi_s[64:128])
                for g in range(4):  # q nope: 4 heads per group
                    ps_qn = [pq2.tile([128, 256], F32, tag=f"pqi{j}",
                                      name=f"ps_qn{j}") for j in range(4)]
                    for qlc in range(12):
                        wqb_t = pw.tile([128, 512], BF16, tag="wqb", name="wqb_t")
                        dma(out=wqb_t[:], in_=wqb[qlc * 128:(qlc + 1) * 128,
                                                  g * 512:(g + 1) * 512])
                        for j in range(4):
                            MM(ps_qn[j][:], wqb_t[:, j * 128:(j + 1) * 128],
                               qaTb[qlc][:], start=(qlc == 0), stop=(qlc == 11))
                    for j in range(4):
                        A.copy(qnT[g * 4 + j][:], ps_qn[j][:])
                for g in range(2):  # q rope pairs: 4 pair-chunks per group
                    ps_qp = [pq2.tile([128, 256], F32, tag=f"pqi{j}",
                                      name=f"ps_qp{j}") for j in range(4)]
                    for qlc in range(12):
                        wqb_t = pw.tile([128, 512], BF16, tag="wqb", name="wqb_t2")
                        dma(out=wqb_t[:], in_=wqb[qlc * 128:(qlc + 1) * 128,
                                                  2048 + g * 512:2048 + (g + 1) * 512])
                        for j in range(4):
                            MM(ps_qp[j][:], wqb_t[:, j * 128:(j + 1) * 128],
                               qaTb[qlc][:], start=(qlc == 0), stop=(qlc == 11))
                    for j in range(4):
                        jj = g * 4 + j
                        qp_s = px.tile([128, 256], F32, tag="qp_s", name="qp_s",
                                       bufs=2)
                        V.tensor_copy(qp_s[:], ps_qp[j][:])
                        for half in range(2):
                            h = jj * 2 + half
                            if half == 0:
                                src = qp_s[0:64]
                            else:
                                st = px.tile([64, 256], F32, tag="qps",
                                             name="qps", bufs=2)
                                dma(out=st[:], in_=qp_s[64:128])
                                src = st[:]
                            qo = px.tile([64, 256], F32, tag="qpo", name="qpo",
                                         bufs=2)
                            rope(qo[:], src, cosq_sb[:], sinq_sb[:])
                            V.tensor_copy(qpeT[h][:], qo[:])

            if debug:
                dma(out=dbg["ki"][:, :], in_=ki_full[:, :])
                dma(out=dbg["kpe"][:, :], in_=kpe_full[:, :])
                dma(out=dbg["kt"][:, :], in_=kT_full[:, :])
                dma(out=dbg["vt"][:, :], in_=v_full[:, :])
                dma(out=dbg["qi3"][:, :], in_=qiTw[3][:])

            # ================= PHASE I: indexer + topk =================
            kisb = [pc.tile([128, 256], F32, tag=f"kisb{b}", name=f"kisb{b}")
                    for b in range(8)]
            kpeb = [pc.tile([64, 256], BF16, tag=f"kpeb{b}", name=f"kpeb{b}")
                    for b in range(8)]
            for b in range(8):
                dma(out=kisb[b][:], in_=ki_full[b * ID:(b + 1) * ID, :])
                kp = px.tile([64, 256], F32, tag="kp", name="kp", bufs=2)
                dma(out=kp[:], in_=kpe_full[b * 64:(b + 1) * 64, :])
                A.copy(kpeb[b][:], kp[:])
            idx_acc = [pc.tile([128, EXTS[t]], F32, tag=f"idx{t}", name=f"idx{t}")
                       for t in range(2)]
            sel = [pc.tile([128, EXTS[t]], BF16, tag=f"sel{t}", name=f"sel{t}")
                   for t in range(2)]
            junk = pc.tile([128, 2048], BF16, tag="junk")
            with tc.tile_pool(name="ppI", bufs=3, space="PSUM") as pi:
                for t in range(2):
                    ext = EXTS[t]
                    for b in range(ext // 256):
                        acc_sl = idx_acc[t][:, b * 256:(b + 1) * 256]
                        for h in range(IH):
                            ps_sh = pi.tile([128, 256], F32, tag="psh")
                            MM(ps_sh[:], qiTw[h][:, t * 128:(t + 1) * 128],
                               kisb[b][:], start=True, stop=True)
                            rl = px.tile([128, 256], F32, tag="rl", bufs=3,
                                         name="rl")
                            A.activation(rl[:], ps_sh[:], AF.Relu)
                            if h == 0:
                                V.tensor_scalar(acc_sl, rl[:],
                                                w_rm[t][:, 0:1], None,
                                                op0=OP.mult)
                            else:
                                V.scalar_tensor_tensor(acc_sl, rl[:],
                                                       w_rm[t][:, h:h + 1],
                                                       acc_sl, op0=OP.mult,
                                                       op1=OP.add)
                    if t == 0:
                        V.tensor_add(idx_acc[0][:], idx_acc[0][:], causal_sb[:])
                    else:
                        V.tensor_add(idx_acc[1][:, 1024:2048],
                                     idx_acc[1][:, 1024:2048], causal_sb[:])
                    lo = px.tile([128, 1], F32, tag="lo", bufs=1, name="lo")
                    hi = px.tile([128, 1], F32, tag="hi", bufs=1, name="hi")
                    mid = px.tile([128, 1], F32, tag="mid", bufs=1, name="mid")
                    cnt = px.tile([128, 1], F32, tag="cnt", bufs=1, name="cnt")
                    ge = px.tile([128, 1], mybir.dt.uint8, tag="ge", bufs=1, name="ge")
                    V.tensor_reduce(lo[:], idx_acc[t][:, 0:512],
                                    axis=mybir.AxisListType.X, op=OP.min)
                    V.tensor_reduce(hi[:], idx_acc[t][:],
                                    axis=mybir.AxisListType.X, op=OP.max)
                    V.tensor_scalar(mid[:], hi[:], -64.0, None, op0=OP.add)
                    V.tensor_max(lo[:], lo[:], mid[:])
                    for it in range(BISECT_ITERS):
                        V.tensor_add(mid[:], lo[:], hi[:])
                        V.tensor_scalar_mul(mid[:], mid[:], 0.5)
                        V.tensor_scalar(junk[:, :ext], idx_acc[t][:], mid[:], 0.0,
                                        op0=OP.is_ge, op1=OP.add,
                                        accum_out=cnt[:])
                        V.tensor_scalar(ge[:], cnt[:], float(TOPK), None,
                                        op0=OP.is_ge)
                        V.copy_predicated(lo[:], ge[:], mid[:])
                        V.tensor_scalar(ge[:], cnt[:], float(TOPK), None,
                                        op0=OP.is_lt)
                        V.copy_predicated(hi[:], ge[:], mid[:])
                    V.tensor_scalar(lo[:], lo[:], -1e29, None, op0=OP.max)
                    V.tensor_scalar(sel[t][:], idx_acc[t][:], lo[:], None,
                                    op0=OP.is_ge)
                    if debug and t == 1:
                        dma(out=dbg["idx1"][:, :], in_=idx_acc[1][:])
                        dma(out=dbg["lo1"][:, :], in_=lo[:])
                        dma(out=dbg["sel1"][:, :], in_=sel[1][:])

            # ================= PHASE A: attention =================
            ones_b = pc.tile([128, 1], BF16, tag="ones_b")
            A.copy(ones_b[:], ones_c_sb[:])
            attnT = [[pc.tile([128, 128], BF16, tag=f"attnT{t}_{h}",
                              name=f"attnT{t}_{h}") for h in range(NH)]
                     for t in range(2)]
            with tc.tile_pool(name="ppA", bufs=2, space="PSUM") as pa:
                for t in range(2):
                    ext = EXTS[t]
                    nch = ext // 128
                    for h in range(NH):
                        out_ps = pa.tile([128, 128], F32, tag="pov")
                        den_p = px.tile([128, 16], F32, tag="denp", bufs=2,
                                        name="den_p")
                        for sc in range(nch):
                            b, hf = sc // 2, sc % 2
                            kt_t = pw.tile([128, 128], BF16, tag="ktl",
                                           name="kt_t", bufs=4)
                            dma(out=kt_t[:],
                                in_=kT_full[b * 2048 + h * 128:
                                            b * 2048 + (h + 1) * 128,
                                            hf * 128:(hf + 1) * 128])
                            v_t = pw.tile([128, 128], BF16, tag="vtl",
                                          name="v_t", bufs=4)
                            dma(out=v_t[:],
                                in_=v_full[b * 256 + hf * 128:
                                           b * 256 + (hf + 1) * 128,
                                           h * 128:(h + 1) * 128])
                            qk_ps = pa.tile([128, 128], F32, tag="pqk")
                            MM(qk_ps[:], qnT[h][:, t * 128:(t + 1) * 128],
                               kt_t[:], start=True, stop=False)
                            MM(qk_ps[:], qpeT[h][:, t * 128:(t + 1) * 128],
                               kpeb[b][:, hf * 128:(hf + 1) * 128],
                               start=False, stop=True)
                            p_t = px.tile([128, 128], BF16, tag="pt", bufs=3,
                                          name="p_t")
                            A.activation(p_t[:], qk_ps[:], AF.Exp, scale=SCALE)
                            pm_t = px.tile([128, 128], BF16, tag="pmt", bufs=3,
                                           name="pm_t")
                            V.scalar_tensor_tensor(
                                pm_t[:], p_t[:], 1.0,
                                sel[t][:, sc * 128:(sc + 1) * 128],
                                op0=OP.bypass, op1=OP.mult,
                                accum_out=den_p[:, sc:sc + 1])
                            tp_ps = pa.tile([128, 128], BF16, tag="ptp")
                            nc.tensor.transpose(tp_ps[:], pm_t[:], ident_sb[:])
                            pT_t = px.tile([128, 128], BF16, tag="pTt", bufs=3,
                                           name="pT_t")
                            A.copy(pT_t[:], tp_ps[:])
                            MM(out_ps[:], pT_t[:], v_t[:],
                               start=(sc == 0), stop=(sc == nch - 1))
                        den = px.tile([128, 1], F32, tag="den", bufs=2, name="den")
                        V.tensor_reduce(den[:], den_p[:, :nch],
                                        axis=mybir.AxisListType.X, op=OP.add)
                        rec = px.tile([128, 1], F32, tag="rec", bufs=2, name="rec")
                        V.reciprocal(rec[:], den[:])
                        o_sb = px.tile([128, 128], BF16, tag="osb", bufs=2,
                                       name="o_sb")
                        V.tensor_scalar(o_sb[:], out_ps[:], rec[:], None,
                                        op0=OP.mult)
                        ot_ps = pa.tile([128, 128], BF16, tag="potp")
                        nc.tensor.transpose(ot_ps[:], o_sb[:], ident_sb[:])
                        A.copy(attnT[t][h][:], ot_ps[:])
            if debug:
                at = px.tile([128, 128], F32, tag="atf32", bufs=1, name="at")
                V.tensor_copy(at[:], attnT[1][0][:])
                dma(out=dbg["att10"][:, :], in_=at[:])

            # ================= PHASE Y: output projection =================
            with tc.tile_pool(name="ppY", bufs=1, space="PSUM") as py:
                ps_y = [[py.tile([128, 512], F32, tag=f"py{t}_{n}",
                                 name=f"psy{t}{n}") for n in range(4)]
                        for t in range(2)]
                for h in range(NH):
                    wo_t = pw.tile([128, 2048], BF16, tag="wo", name="wo_t", bufs=2)
                    dma(out=wo_t[:], in_=wo[h * 128:(h + 1) * 128, :])
                    for t in range(2):
                        for n in range(4):
                            MM(ps_y[t][n][:], attnT[t][h][:],
                               wo_t[:, n * 512:(n + 1) * 512],
                               start=(h == 0), stop=(h == NH - 1))
                for t in range(2):
                    for n in range(4):
                        y_sb = px.tile([128, 512], F32, tag="ysb", bufs=2,
                                       name="y_sb")
                        V.tensor_copy(y_sb[:], ps_y[t][n][:])
                        dma(out=y[t][:, n * 512:(n + 1) * 512], in_=y_sb[:])

    nc.compile()
    return nc


def _host_prep(inputs):
    """Build shared + per-core input arrays."""
    hs = np.asarray(inputs["hidden_states"], np.float32)[0]
    hsT = np.ascontiguousarray(hs.T)
    cosT = np.asarray(inputs["cos"], np.float32).T
    sinT = np.asarray(inputs["sin"], np.float32).T
    cosT2 = np.ascontiguousarray(np.vstack([cosT, cosT]))
    sinT2 = np.ascontiguousarray(np.vstack([-sinT, sinT]))
    Wq_b = np.asarray(inputs["Wq_b"], np.float32)
    cols = [Wq_b[:, h * 192:h * 192 + 128] for h in range(NH)]
    for j in range(8):
        cols.append(Wq_b[:, (2 * j) * 192 + 128:(2 * j) * 192 + 192])
        cols.append(Wq_b[:, (2 * j + 1) * 192 + 128:(2 * j + 1) * 192 + 192])
    wqb_re = np.ascontiguousarray(np.concatenate(cols, 1)).astype(BF)

    shared = {
        "wqa": np.ascontiguousarray(inputs["Wq_a"], dtype=np.float32),
        "qga": np.ascontiguousarray(
            np.asarray(inputs["q_a_gamma"], np.float32).reshape(12, 128).T),
        "wqb": wqb_re,
        "wkva": np.asarray(inputs["Wkv_a"], np.float32).astype(BF),
        "kvga": np.ascontiguousarray(
            np.asarray(inputs["kv_a_gamma"], np.float32).reshape(4, 128).T),
        "wkvb": np.asarray(inputs["Wkv_b"], np.float32).astype(BF),
        "wo": np.asarray(inputs["Wo"], np.float32).astype(BF),
        "wqi": np.ascontiguousarray(inputs["Wq_idx"], dtype=np.float32),
        "wki": np.ascontiguousarray(inputs["Wk_idx"], dtype=np.float32),
        "wwi": np.ascontiguousarray(
            np.asarray(inputs["Ww_idx"], np.float32) * (ID ** -0.5)),
        "knga": np.ascontiguousarray(
            np.asarray(inputs["kn_gamma"], np.float32).reshape(128, 1)),
        "knbe": np.ascontiguousarray(
            np.asarray(inputs["kn_beta"], np.float32).reshape(128, 1)),
        "ones_c": np.ones((128, 1), np.float32),
        "cnsts": np.tile(np.array([[0.0, EPS]], np.float32), (128, 1)),
        "ones_r": np.ones((1, 128), np.float32),
        "ident": np.eye(128, dtype=np.float32).astype(BF),
    }
    in_maps = []
    rows_all = []
    for c in range(NC):
        r0 = c + 8 * np.arange(128)
        r1 = 1024 + c + 8 * np.arange(128)
        rows = np.concatenate([r0, r1])
        rows_all.append((r0, r1))
        kvs = slice(c * 256, (c + 1) * 256)
        m = dict(shared)
        m["hsq"] = np.ascontiguousarray(hsT[:, rows])
        m["hskv"] = np.ascontiguousarray(hsT[:, kvs])
        m["hskvb"] = m["hskv"].astype(BF)
        m["cosq"] = np.ascontiguousarray(cosT2[:, rows])
        m["sinq"] = np.ascontiguousarray(sinT2[:, rows])
        m["coskv"] = np.ascontiguousarray(cosT2[:, kvs])
        m["sinkv"] = np.ascontiguousarray(sinT2[:, kvs])
        m["causal"] = np.ascontiguousarray(
            np.where(np.arange(1024)[None, :] <= (c + 8 * np.arange(128))[:, None],
                     0.0, -1e30).astype(np.float32))
        in_maps.append(m)
    return in_maps, rows_all


def kernel(hidden_states, cos, sin, Wq_a, q_a_gamma, Wq_b, Wkv_a, kv_a_gamma,
           Wkv_b, Wo, Wq_idx, Wk_idx, Ww_idx, kn_gamma, kn_beta, topk):
    from concourse.bass_utils import run_bass_kernel_spmd

    assert int(topk) == TOPK
    inputs = dict(hidden_states=hidden_states, cos=cos, sin=sin, Wq_a=Wq_a,
                  q_a_gamma=q_a_gamma, Wq_b=Wq_b, Wkv_a=Wkv_a,
                  kv_a_gamma=kv_a_gamma, Wkv_b=Wkv_b, Wo=Wo, Wq_idx=Wq_idx,
                  Wk_idx=Wk_idx, Ww_idx=Ww_idx, kn_gamma=kn_gamma,
                  kn_beta=kn_beta)
    debug = bool(int(os.environ.get("BASS_DEBUG", "0")))
    if "nc" not in _cached or _cached.get("debug") != debug:
        _cached["nc"] = _build(debug=debug)
        _cached["debug"] = debug
    nc = _cached["nc"]
    in_maps, rows_all = _host_prep(inputs)
    res = run_bass_kernel_spmd(nc, in_maps, list(range(NC)))
    _cached["last_res"] = res
    y = np.empty((S, H), np.float32)
    for c in range(NC):
        r0, r1 = rows_all[c]
        yt = res.results[c]["y"]
        y[r0] = yt[0]
        y[r1] = yt[1]
    return y[None]
